# revision 1
# baseline (speedup 1.0000x reference)
"""DigitCaps dynamic-routing kernel for 8 Trainium2 NeuronCores.

Strategy: shard the num_route_nodes axis (R=2048 -> 256 per core).
  - Phase 1: u_hat production. Per route r: u[b, (c,m)] = xT_r[k,b].T @ w_r[k,(c,m)]
    on the tensor engine (fp32). u staged in device DRAM; the first routing
    iteration (c uniform = 1/CAPS) is fused in as a running sum over routes.
  - Phase 2: each remaining routing iteration is ONE streaming pass over u:
    per r-tile: dot = sum_m u*v  ->  b_logits += dot -> softmax over caps
    (tile-local) -> s_partial += sum_r c*u.  s is AllReduced across cores
    (contraction over r spans cores), squash computed redundantly per core.

Inputs are sharded host-side: x -> xT[k, r_loc, b] slices, w -> w[r_loc, k, c, m]
slices (transpose is layout prep for DMA/matmul efficiency; all FLOPs on device).
"""

import os
import sys

if "/opt/trn_rl_repo" not in sys.path:
    sys.path.insert(0, "/opt/trn_rl_repo")

import numpy as np

B, R, K, C, M = 128, 2048, 64, 32, 32
CM = C * M
N_CORES = 8
R_LOC = R // N_CORES
RT1 = int(os.environ.get("DC_RT1", "8"))   # routes per tile, u-production
RT2 = int(os.environ.get("DC_RT2", "8"))  # routes per tile, routing passes
S1_ON_PE = os.environ.get("DC_S1PE", "1") == "1"
SIM_MODE = os.environ.get("DC_SIM", "0") == "1"   # 1-core, collective->copy
V_EXP = os.environ.get("DC_VEXP", "1") == "1"     # materialize v expanded
C_EXP = os.environ.get("DC_CEXP", "0") == "1"     # materialize coef expanded (ACT)
Q_SWAP = os.environ.get("DC_QSWAP", "0") == "1"   # coef as in0 in the q product

PROD_ENGINE = os.environ.get("DC_PROD", "vector")   # "vector" | "gpsimd"
U_DT = os.environ.get("DC_U_DT", "float16")         # staged-u dtype
MM_DT = os.environ.get("DC_MM", "float32r")         # matmul input dtype

_compiled = {}
LAST_RESULT = None          # BassKernelResults of the most recent run (for test.py)


def _view(ap, dims):
    """Free-dim view of an AP: keep its partition dim, replace free dims by
    [step, count] pairs (element steps). step 0 = broadcast."""
    import concourse.bass as bass

    return bass.AP(
        tensor=ap.tensor,
        offset=ap.offset,
        ap=[list(ap.ap[0])] + [[s, c] for s, c in dims],
    )


def _ap(ap, dims):
    """Fully custom AP (all dims given) at the base offset of `ap`."""
    import concourse.bass as bass

    return bass.AP(
        tensor=ap.tensor,
        offset=ap.offset,
        ap=[[s, c] for s, c in dims],
    )


def _squash(nc, pool, s_ap, v_ap):
    """v = s * |s|^2 / ((1 + |s|^2) (sqrt(|s|^2) + 1e-8)), norm over m."""
    import concourse.mybir as mybir

    f32 = mybir.dt.float32
    op = mybir.AluOpType
    sq_full = pool.tile([B, CM], f32, tag="sq_full")
    nc.vector.tensor_tensor(sq_full[:], s_ap, s_ap, op=op.mult)
    sq = pool.tile([B, C], f32, tag="sq")
    nc.vector.tensor_reduce(
        sq[:], _view(sq_full[:], [(1, C), (C, M)]), axis=mybir.AxisListType.X,
        op=op.add)
    rt = pool.tile([B, C], f32, tag="rt")
    nc.scalar.activation(rt[:], sq[:], mybir.ActivationFunctionType.Sqrt)
    nc.vector.tensor_scalar(rt[:], rt[:], 1e-8, None, op0=op.add)
    den = pool.tile([B, C], f32, tag="den")
    nc.vector.tensor_scalar(den[:], sq[:], 1.0, None, op0=op.add)
    nc.vector.tensor_tensor(den[:], den[:], rt[:], op=op.mult)
    fi = pool.tile([B, C], f32, tag="fi")
    nc.vector.reciprocal(fi[:], den[:])
    nc.vector.tensor_tensor(fi[:], fi[:], sq[:], op=op.mult)
    # v = s * f (f broadcast over m)
    nc.vector.tensor_tensor(
        v_ap,
        _view(s_ap, [(C, M), (1, C)]),
        _view(fi[:], [(0, M), (1, C)]),
        op=op.mult,
    )


def _build(n_iters, repeat=1):
    import concourse.mybir as mybir
    import concourse.tile as tile
    from concourse import bacc

    f32 = mybir.dt.float32
    u_dt = getattr(mybir.dt, U_DT)
    mm_dt = getattr(mybir.dt, MM_DT)
    op = mybir.AluOpType
    AX = mybir.AxisListType

    nc = bacc.Bacc("TRN2", target_bir_lowering=False, debug=False,
                   num_devices=1 if SIM_MODE else N_CORES)
    xT = nc.dram_tensor("xT", [R_LOC // 2, 2, K, B], mm_dt,
                        kind="ExternalInput").ap()
    wT = nc.dram_tensor("wT", [R_LOC // 2, 2, K, CM], mm_dt,
                        kind="ExternalInput").ap()
    out = nc.dram_tensor("out", [B, CM], f32, kind="ExternalOutput").ap()

    if PROD_ENGINE == "split":
        prod_p, prod_q = nc.vector, nc.gpsimd
    else:
        prod_p = prod_q = {"gpsimd": nc.gpsimd, "vector": nc.vector}[PROD_ENGINE]

    with tile.TileContext(nc) as tc:
        with (
            tc.tile_pool(name="sm", bufs=2) as sm,       # small temps
            tc.tile_pool(name="persist", bufs=1) as persist,
            tc.tile_pool(name="dram", bufs=1, space="DRAM") as dram,
            tc.tile_pool(name="drbounce", bufs=min(2 * n_iters * repeat, 8),
                         space="DRAM") as drb,
        ):
            u_dram = dram.tile([B, R_LOC * CM], u_dt)
            b_log = persist.tile([B, R_LOC * C], f32)   # logits, layout (r, c)
            v_sb = persist.tile([B, CM], f32)           # current v (fp32)
            if u_dt != f32:
                v_u = persist.tile([B, CM], u_dt, tag="v_u")
            else:
                v_u = v_sb

            def allreduce_squash(s_acc_tile, scale):
                bin_ = drb.tile([B, CM], f32, tag="bin")
                bout = drb.tile([B, CM], f32, tag="bout")
                nc.sync.dma_start(bin_[:], s_acc_tile[:])
                if SIM_MODE:
                    nc.sync.dma_start(bout[:], bin_[:])
                else:
                    nc.gpsimd.collective_compute(
                        "AllReduce", op.add,
                        replica_groups=[list(range(N_CORES))],
                        ins=[bin_.opt()], outs=[bout.opt()],
                    )
                s_sb = sm.tile([B, CM], f32, tag="s_sb")
                nc.sync.dma_start(s_sb[:], bout[:])
                if scale != 1.0:
                    nc.vector.tensor_scalar(s_sb[:], s_sb[:], scale, None,
                                            op0=op.mult)
                _squash(nc, sm, s_sb[:], v_sb[:])
                if not V_EXP and v_u is not v_sb:
                    nc.vector.tensor_copy(v_u[:], v_sb[:])

            def emit_phase1_packed():
                """u production with route-pairs packed on 128 partitions;
                iteration-1 s accumulated on the PE in a dedicated PSUM pair
                via K=128 packed matmuls (u_r0 + u_r1 per pair)."""
                s_acc = sm.tile([B, CM], f32, tag="s_acc")
                n_tiles = R_LOC // RT1
                half = RT1 // 2
                with (
                    tc.tile_pool(name="xp", bufs=3) as xp,
                    tc.tile_pool(name="wp", bufs=3) as wp,
                    tc.tile_pool(name="up1", bufs=3) as up1,
                    tc.tile_pool(name="pp", bufs=3, space="PSUM") as pp,
                    tc.tile_pool(name="s1p", bufs=1, space="PSUM") as s1p,
                ):
                    s1_psum = s1p.tile([B, CM], f32)
                    for t in range(n_tiles):
                        xt = xp.tile([2 * K, half * B], mm_dt)
                        # partition p = k + 64*(r%2); host layout
                        # [rp, par, k, ...] makes (par, k) one stride run
                        nc.sync.dma_start(
                            xt[:],
                            _ap(xT[t * half:(t + 1) * half],
                                [(B, 2 * K), (2 * K * B, half), (1, B)]))
                        wt = wp.tile([2 * K, half * CM], mm_dt)
                        nc.sync.dma_start(
                            wt[:],
                            _ap(wT[t * half:(t + 1) * half],
                                [(CM, 2 * K), (2 * K * CM, half), (1, CM)]))
                        ut = up1.tile([B, RT1 * CM], u_dt)
                        for rp in range(half):
                            first = (t == 0 and rp == 0)
                            last = (t == n_tiles - 1 and rp == half - 1)
                            for h in range(2):
                                # packed: u_even + u_odd accumulated into s1
                                nc.tensor.matmul(
                                    s1_psum[:, h * 512:(h + 1) * 512],
                                    xt[:, rp * B:(rp + 1) * B],
                                    wt[:, rp * CM + h * 512:
                                       rp * CM + (h + 1) * 512],
                                    start=first, stop=last,
                                )
                            for par in range(2):
                                j = 2 * rp + par
                                ps = pp.tile([B, CM], f32)
                                for h in range(2):
                                    nc.tensor.matmul(
                                        ps[:, h * 512:(h + 1) * 512],
                                        xt[par * K:(par + 1) * K,
                                           rp * B:(rp + 1) * B],
                                        wt[par * K:(par + 1) * K,
                                           rp * CM + h * 512:
                                           rp * CM + (h + 1) * 512],
                                        start=True, stop=True,
                                    )
                                if j % 2 == 0:
                                    nc.scalar.copy(
                                        ut[:, j * CM:(j + 1) * CM], ps[:])
                                else:
                                    nc.vector.tensor_copy(
                                        ut[:, j * CM:(j + 1) * CM], ps[:])
                        nc.sync.dma_start(
                            u_dram[:, t * RT1 * CM:(t + 1) * RT1 * CM], ut[:])
                    nc.vector.tensor_copy(s_acc[:], s1_psum[:])
                return s_acc

            def emit_phase1_plain():
                s_acc = sm.tile([B, CM], f32, tag="s_acc")
                nc.vector.memset(s_acc[:], 0.0)
                with (
                    tc.tile_pool(name="xp", bufs=3) as xp,
                    tc.tile_pool(name="wp", bufs=3) as wp,
                    tc.tile_pool(name="up1", bufs=3) as up1,
                    tc.tile_pool(name="pp", bufs=4, space="PSUM") as pp,
                ):
                    for t in range(R_LOC // RT1):
                        xt = xp.tile([K, RT1 * B], mm_dt)
                        nc.sync.dma_start(
                            xt[:],
                            _ap(xT[t * RT1 // 2:(t + 1) * RT1 // 2],
                                [(B, K), (K * B, RT1), (1, B)]))
                        wt = wp.tile([K, RT1 * CM], mm_dt)
                        nc.sync.dma_start(
                            wt[:],
                            _ap(wT[t * RT1 // 2:(t + 1) * RT1 // 2],
                                [(CM, K), (K * CM, RT1), (1, CM)]),
                        )
                        ut = up1.tile([B, RT1 * CM], u_dt)
                        for j in range(RT1):
                            ps = pp.tile([B, CM], f32)
                            for h in range(2):
                                nc.tensor.matmul(
                                    ps[:, h * 512:(h + 1) * 512],
                                    xt[:, j * B:(j + 1) * B],
                                    wt[:, j * CM + h * 512:
                                       j * CM + (h + 1) * 512],
                                    start=True, stop=True,
                                )
                            nc.scalar.copy(ut[:, j * CM:(j + 1) * CM], ps[:])
                        # s1 partial: sum over the tile's routes
                        red = sm.tile([B, CM], f32, tag="red")
                        nc.vector.tensor_reduce(
                            red[:], _view(ut[:], [(1, CM), (CM, RT1)]),
                            axis=AX.X, op=op.add)
                        nc.vector.tensor_tensor(s_acc[:], s_acc[:], red[:],
                                                op=op.add)
                        nc.sync.dma_start(
                            u_dram[:, t * RT1 * CM:(t + 1) * RT1 * CM], ut[:])
                return s_acc

            def emit_once():
                # ------------- Phase 1: u production + iteration-1 s ---------
                if S1_ON_PE:
                    s_acc = emit_phase1_packed()
                else:
                    s_acc = emit_phase1_plain()

                allreduce_squash(s_acc, 1.0 / C)

                # ------------- Phase 2: remaining routing iterations ---------
                with (
                    tc.tile_pool(name="up2",
                                 bufs=int(os.environ.get("DC_UPB", "2"))) as up2,
                    tc.tile_pool(name="pq",
                                 bufs=int(os.environ.get("DC_PQB", "2"))) as pq,
                    tc.tile_pool(name="vxp", bufs=1) as vxp,
                    tc.tile_pool(name="cxp", bufs=2) as cxp,
                ):
                    for it in range(2, n_iters + 1):
                        s_acc = sm.tile([B, CM], f32, tag="s_acc")
                        if V_EXP:
                            # v expanded over tile routes (ACT, once/pass)
                            v_exp = vxp.tile([B, RT2 * CM], u_dt, tag="v_exp")
                            nc.scalar.copy(
                                v_exp[:], _view(v_sb[:], [(0, RT2), (1, CM)]))
                        for t in range(R_LOC // RT2):
                            ut = up2.tile([B, RT2 * CM], u_dt)
                            nc.sync.dma_start(
                                ut[:],
                                u_dram[:, t * RT2 * CM:(t + 1) * RT2 * CM])
                            p = pq.tile([B, RT2 * CM], u_dt, tag="pq")
                            if V_EXP:
                                # both contiguous -> DVE 2x mode
                                prod_p.tensor_tensor(p[:], ut[:], v_exp[:],
                                                     op=op.mult)
                            else:
                                prod_p.tensor_tensor(
                                    _view(p[:], [(CM, RT2), (C, M), (1, C)]),
                                    _view(ut[:], [(CM, RT2), (C, M), (1, C)]),
                                    _view(v_u[:], [(0, RT2), (C, M), (1, C)]),
                                    op=op.mult)
                            # dot[b, (r_t, c)] = sum_m p
                            blt = b_log[:, t * RT2 * C:(t + 1) * RT2 * C]
                            if it == 2:
                                nc.vector.tensor_reduce(
                                    blt,
                                    _view(p[:], [(CM, RT2), (1, C), (C, M)]),
                                    axis=AX.X, op=op.add)
                            else:
                                dot = sm.tile([B, RT2 * C], f32, tag="dot")
                                nc.vector.tensor_reduce(
                                    dot[:],
                                    _view(p[:], [(CM, RT2), (1, C), (C, M)]),
                                    axis=AX.X, op=op.add)
                                nc.vector.tensor_tensor(blt, blt, dot[:],
                                                        op=op.add)
                            # softmax over caps (innermost c of blt)
                            mx = sm.tile([B, RT2], f32, tag="mx")
                            nc.vector.tensor_reduce(
                                mx[:], _view(blt, [(C, RT2), (1, C)]),
                                axis=AX.X, op=op.max)
                            e = sm.tile([B, RT2 * C], f32, tag="e")
                            nc.vector.tensor_tensor(
                                _view(e[:], [(C, RT2), (1, C)]),
                                _view(blt, [(C, RT2), (1, C)]),
                                _view(mx[:], [(1, RT2), (0, C)]),
                                op=op.subtract)
                            nc.scalar.activation(
                                e[:], e[:], mybir.ActivationFunctionType.Exp)
                            z = sm.tile([B, RT2], f32, tag="z")
                            nc.vector.tensor_reduce(
                                z[:], _view(e[:], [(C, RT2), (1, C)]),
                                axis=AX.X, op=op.add)
                            nc.vector.reciprocal(z[:], z[:])
                            coef = sm.tile([B, RT2 * C], u_dt, tag="coef")
                            nc.vector.tensor_tensor(
                                _view(coef[:], [(C, RT2), (1, C)]),
                                _view(e[:], [(C, RT2), (1, C)]),
                                _view(z[:], [(1, RT2), (0, C)]),
                                op=op.mult)
                            q = pq.tile([B, RT2 * CM], u_dt, tag="pq")
                            if C_EXP:
                                coef_exp = cxp.tile([B, RT2 * CM], u_dt,
                                                    tag="coef_exp")
                                nc.scalar.copy(
                                    coef_exp[:],
                                    _view(coef[:],
                                          [(C, RT2), (0, M), (1, C)]))
                                prod_q.tensor_tensor(q[:], ut[:],
                                                     coef_exp[:], op=op.mult)
                            elif Q_SWAP:
                                prod_q.tensor_tensor(
                                    _view(q[:], [(CM, RT2), (C, M), (1, C)]),
                                    _view(coef[:],
                                          [(C, RT2), (0, M), (1, C)]),
                                    _view(ut[:], [(CM, RT2), (C, M), (1, C)]),
                                    op=op.mult)
                            else:
                                prod_q.tensor_tensor(
                                    _view(q[:], [(CM, RT2), (C, M), (1, C)]),
                                    _view(ut[:], [(CM, RT2), (C, M), (1, C)]),
                                    _view(coef[:],
                                          [(C, RT2), (0, M), (1, C)]),
                                    op=op.mult)
                            # s partial += sum over r_t of q
                            if t == 0:
                                nc.vector.tensor_reduce(
                                    s_acc[:],
                                    _view(q[:], [(C, M), (1, C), (CM, RT2)]),
                                    axis=AX.X, op=op.add)
                            else:
                                red = sm.tile([B, CM], f32, tag="red")
                                nc.vector.tensor_reduce(
                                    red[:],
                                    _view(q[:], [(C, M), (1, C), (CM, RT2)]),
                                    axis=AX.X, op=op.add)
                                nc.vector.tensor_tensor(s_acc[:], s_acc[:],
                                                        red[:], op=op.add)
                        allreduce_squash(s_acc, 1.0)

            for _ in range(repeat):
                emit_once()

            nc.sync.dma_start(out[:], v_sb[:])

    nc.compile()
    return nc


def kernel(x, route_weights, num_iterations):
    global LAST_RESULT
    from concourse import bass_utils

    n = int(num_iterations)
    assert n >= 1
    x = np.asarray(x, dtype=np.float32)
    w = np.asarray(route_weights, dtype=np.float32)
    assert x.shape == (B, R, K) and w.shape == (R, C, K, M)

    if n not in _compiled:
        _compiled[n] = _build(n)
    nc = _compiled[n]

    in_maps = []
    for c in range(N_CORES):
        sl = slice(c * R_LOC, (c + 1) * R_LOC)
        xT_c = np.ascontiguousarray(
            x[:, sl, :].transpose(1, 2, 0).reshape(R_LOC // 2, 2, K, B))
        wT_c = np.ascontiguousarray(
            w[sl].reshape(R_LOC // 2, 2, C, K, M).transpose(0, 1, 3, 4, 2)
        ).reshape(R_LOC // 2, 2, K, CM)
        in_maps.append({"xT": xT_c, "wT": wT_c})

    res = bass_utils.run_bass_kernel_spmd(
        nc, in_maps, core_ids=list(range(N_CORES)))
    LAST_RESULT = res
    return np.ascontiguousarray(
        res.results[0]["out"].reshape(B, M, C).transpose(0, 2, 1)
    ).astype(np.float32)



# revision 35
# speedup vs baseline: 56.4551x; 56.4551x over previous
"""DigitCaps dynamic-routing kernel for 8 Trainium2 NeuronCores.

Strategy: shard the num_route_nodes axis (R=2048 -> 256 per core).
  - Phase 1: u_hat production. Per route r: u[b, (c,m)] = xT_r[k,b].T @ w_r[k,(c,m)]
    on the tensor engine (fp32). u staged in device DRAM; the first routing
    iteration (c uniform = 1/CAPS) is fused in as a running sum over routes.
  - Phase 2: each remaining routing iteration is ONE streaming pass over u:
    per r-tile: dot = sum_m u*v  ->  b_logits += dot -> softmax over caps
    (tile-local) -> s_partial += sum_r c*u.  s is AllReduced across cores
    (contraction over r spans cores), squash computed redundantly per core.

Inputs are sharded host-side: x -> xT[k, r_loc, b] slices, w -> w[r_loc, k, c, m]
slices (transpose is layout prep for DMA/matmul efficiency; all FLOPs on device).
"""

import os
import sys

if "/opt/trn_rl_repo" not in sys.path:
    sys.path.insert(0, "/opt/trn_rl_repo")

import numpy as np

B, R, K, C, M = 128, 2048, 64, 32, 32
CM = C * M
N_CORES = 8
R_LOC = R // N_CORES
RT1 = int(os.environ.get("DC_RT1", "8"))   # routes per tile, u-production
RT2 = int(os.environ.get("DC_RT2", "8"))  # routes per tile, routing passes
S1_ON_PE = os.environ.get("DC_S1PE", "1") == "1"
SIM_MODE = os.environ.get("DC_SIM", "0") == "1"   # 1-core, collective->copy
V_EXP = os.environ.get("DC_VEXP", "0") == "1"     # materialize v expanded
C_EXP = os.environ.get("DC_CEXP", "0") == "1"     # materialize coef expanded (ACT)
Q_SWAP = os.environ.get("DC_QSWAP", "0") == "1"   # coef as in0 in the q product

PROD_ENGINE = os.environ.get("DC_PROD", "vector")   # "vector" | "gpsimd"
U_DT = os.environ.get("DC_U_DT", "float16")         # staged-u dtype
MM_DT = os.environ.get("DC_MM", "float16")         # matmul input dtype
TREE = os.environ.get("DC_TREE", "1") == "1"        # log-tree reductions (2x DVE)
TREE16 = int(os.environ.get("DC_TREE16", "2"))      # fp16 rounds in the dot tree
P1COPY = os.environ.get("DC_P1COPY", "scalar")      # phase-1 psum->sbuf engine
RECOMP = os.environ.get("DC_RECOMP", "1") == "1"    # recompute u per pass on PE
PEMM = os.environ.get("DC_PEMM", "1") == "1"        # dot/s reductions as PE matmuls

_compiled = {}
LAST_RESULT = None          # BassKernelResults of the most recent run (for test.py)


def _view(ap, dims):
    """Free-dim view of an AP: keep its partition dim, replace free dims by
    [step, count] pairs (element steps). step 0 = broadcast."""
    import concourse.bass as bass

    return bass.AP(
        tensor=ap.tensor,
        offset=ap.offset,
        ap=[list(ap.ap[0])] + [[s, c] for s, c in dims],
    )


def _ap(ap, dims):
    """Fully custom AP (all dims given) at the base offset of `ap`."""
    import concourse.bass as bass

    return bass.AP(
        tensor=ap.tensor,
        offset=ap.offset,
        ap=[[s, c] for s, c in dims],
    )


def _squash(nc, pool, s_ap, v_ap):
    """v = s * |s|^2 / ((1 + |s|^2) (sqrt(|s|^2) + 1e-8)), norm over m."""
    import concourse.mybir as mybir

    f32 = mybir.dt.float32
    op = mybir.AluOpType
    sq_full = pool.tile([B, CM], f32, tag="sq_full")
    nc.vector.tensor_tensor(sq_full[:], s_ap, s_ap, op=op.mult)
    sq = pool.tile([B, C], f32, tag="sq")
    nc.vector.tensor_reduce(
        sq[:], _view(sq_full[:], [(1, C), (C, M)]), axis=mybir.AxisListType.X,
        op=op.add)
    rt = pool.tile([B, C], f32, tag="rt")
    nc.scalar.activation(rt[:], sq[:], mybir.ActivationFunctionType.Sqrt)
    nc.vector.tensor_scalar(rt[:], rt[:], 1e-8, None, op0=op.add)
    den = pool.tile([B, C], f32, tag="den")
    nc.vector.tensor_scalar(den[:], sq[:], 1.0, None, op0=op.add)
    nc.vector.tensor_tensor(den[:], den[:], rt[:], op=op.mult)
    fi = pool.tile([B, C], f32, tag="fi")
    nc.vector.reciprocal(fi[:], den[:])
    nc.vector.tensor_tensor(fi[:], fi[:], sq[:], op=op.mult)
    # v = s * f (f broadcast over m)
    nc.vector.tensor_tensor(
        v_ap,
        _view(s_ap, [(C, M), (1, C)]),
        _view(fi[:], [(0, M), (1, C)]),
        op=op.mult,
    )


def _tree_dot(nc, sm, p_tile, blt, it):
    """dot[b,(r,c)] = sum_m p[b,(r,m,c)] via log-tree halving of the m extent.
    Layout per route: CM = (m major, c minor). Each round adds the two
    contiguous m-halves of every route segment; first TREE16 rounds keep
    fp16 (DVE 2x mode), the rest accumulate in fp32. Final round writes blt
    (it==2) or adds a dot scratch into blt."""
    import concourse.mybir as mybir

    f32 = mybir.dt.float32
    f16 = mybir.dt.float16
    op = mybir.AluOpType
    cur = p_tile
    seg = CM
    level = 0
    while seg > 2 * C:
        half = seg // 2
        dt = f16 if level < TREE16 else f32
        nxt = sm.tile([B, RT2 * half], dt,
                      tag=f"tr{RT2 * half}{'h' if dt == f16 else 's'}")
        nc.vector.tensor_tensor(
            _view(nxt[:], [(half, RT2), (1, half)]),
            _view(cur[:], [(seg, RT2), (1, half)]),
            _view(cur[:, half:], [(seg, RT2), (1, half)]),
            op=op.add)
        cur = nxt
        seg = half
        level += 1
    # last round: seg == 2*C -> C, always fp32 out
    if it == 2:
        out_ap = _view(blt, [(C, RT2), (1, C)])
    else:
        dot = sm.tile([B, RT2 * C], f32, tag="dot")
        out_ap = _view(dot[:], [(C, RT2), (1, C)])
    nc.vector.tensor_tensor(
        out_ap,
        _view(cur[:], [(seg, RT2), (1, C)]),
        _view(cur[:, C:], [(seg, RT2), (1, C)]),
        op=op.add)
    if it != 2:
        nc.vector.tensor_tensor(blt, blt, dot[:], op=op.add)


def _tree_s(nc, sm, q_tile, s_acc, first):
    """s_partial[b,(m,c)] = sum_r q[b,(r,m,c)]: contiguous half-folds over
    the route extent (all fp16, DVE 2x), then fp32 accumulate into s_acc."""
    import concourse.mybir as mybir

    f16 = mybir.dt.float16
    op = mybir.AluOpType
    cur = q_tile
    rs = RT2
    level = 0
    while rs > 1:
        half = rs // 2
        nxt = sm.tile([B, half * CM], f16, tag=f"tr{half * CM}h")
        nc.vector.tensor_tensor(
            nxt[:], cur[:, :half * CM], cur[:, half * CM:], op=op.add)
        cur = nxt
        rs = half
        level += 1
    if first:
        nc.vector.tensor_copy(s_acc[:], cur[:])
    else:
        nc.vector.tensor_tensor(s_acc[:], s_acc[:], cur[:], op=op.add)


def _build(n_iters, repeat=1):
    import concourse.mybir as mybir
    import concourse.tile as tile
    from concourse import bacc

    f32 = mybir.dt.float32
    u_dt = getattr(mybir.dt, U_DT)
    mm_dt = getattr(mybir.dt, MM_DT)
    op = mybir.AluOpType
    AX = mybir.AxisListType

    nc = bacc.Bacc("TRN2", target_bir_lowering=False, debug=False,
                   num_devices=1 if SIM_MODE else N_CORES)
    xT = nc.dram_tensor("xT", [R_LOC // 2, 2, K, B], mm_dt,
                        kind="ExternalInput").ap()
    wT = nc.dram_tensor("wT", [R_LOC // 2, 2, K, CM], mm_dt,
                        kind="ExternalInput").ap()
    out = nc.dram_tensor("out", [B, CM], f32, kind="ExternalOutput").ap()

    if PROD_ENGINE == "split":
        prod_p, prod_q = nc.vector, nc.gpsimd
    else:
        prod_p = prod_q = {"gpsimd": nc.gpsimd, "vector": nc.vector}[PROD_ENGINE]

    with tile.TileContext(nc) as tc:
        with (
            tc.tile_pool(name="sm", bufs=2) as sm,       # small temps
            tc.tile_pool(name="persist", bufs=1) as persist,
            tc.tile_pool(name="dram", bufs=1, space="DRAM") as dram,
            tc.tile_pool(name="drbounce", bufs=min(2 * n_iters * repeat, 8),
                         space="DRAM") as drb,
        ):
            u_dram = (None if RECOMP and n_iters <= 2
                      else dram.tile([B, R_LOC * CM], u_dt))
            b_log = persist.tile([B, R_LOC * C], f32)   # logits, layout (r, c)
            if PEMM:
                # fp16 identity for PE-side free-dim reductions:
                # out[b, j] (+)= sum_k eye[k, b] * rhs[k, j] selects rhs row b.
                eye = persist.tile([B, B], u_dt, tag="eye")
                ii = persist.tile([B, B], mybir.dt.int32, tag="eye_i")
                jj = persist.tile([B, B], mybir.dt.int32, tag="eye_j")
                nc.gpsimd.iota(ii[:], [[1, B]], base=0, channel_multiplier=0)
                nc.gpsimd.iota(jj[:], [[0, B]], base=0, channel_multiplier=1)
                nc.vector.tensor_tensor(eye[:], ii[:], jj[:], op=op.is_equal)
            v_sb = persist.tile([B, CM], f32)           # current v (fp32)
            if u_dt != f32:
                v_u = persist.tile([B, CM], u_dt, tag="v_u")
            else:
                v_u = v_sb

            def allreduce_squash(s_acc_tile, scale):
                bin_ = drb.tile([B, CM], f32, tag="bin")
                bout = drb.tile([B, CM], f32, tag="bout")
                nc.sync.dma_start(bin_[:], s_acc_tile[:])
                if SIM_MODE:
                    nc.sync.dma_start(bout[:], bin_[:])
                else:
                    nc.gpsimd.collective_compute(
                        "AllReduce", op.add,
                        replica_groups=[list(range(N_CORES))],
                        ins=[bin_.opt()], outs=[bout.opt()],
                    )
                s_sb = sm.tile([B, CM], f32, tag="s_sb")
                nc.sync.dma_start(s_sb[:], bout[:])
                if scale != 1.0:
                    nc.vector.tensor_scalar(s_sb[:], s_sb[:], scale, None,
                                            op0=op.mult)
                _squash(nc, sm, s_sb[:], v_sb[:])
                if not V_EXP and v_u is not v_sb:
                    nc.vector.tensor_copy(v_u[:], v_sb[:])

            def emit_s1_only():
                """Iteration-1 s on the PE only (packed K=128 route-pairs,
                PSUM-accumulated over all local routes). u is NOT materialized
                anywhere -- passes 2..n recompute it tile-by-tile."""
                s_acc = sm.tile([B, CM], f32, tag="s_acc")
                n_tiles = R_LOC // RT1
                half = RT1 // 2
                with (
                    tc.tile_pool(name="xp", bufs=3) as xp,
                    tc.tile_pool(name="wp", bufs=3) as wp,
                    tc.tile_pool(name="s1p", bufs=1, space="PSUM") as s1p,
                ):
                    s1_psum = s1p.tile([B, CM], f32)
                    for t in range(n_tiles):
                        xt = xp.tile([2 * K, half * B], mm_dt)
                        nc.sync.dma_start(
                            xt[:],
                            _ap(xT[t * half:(t + 1) * half],
                                [(B, 2 * K), (2 * K * B, half), (1, B)]))
                        wt = wp.tile([2 * K, half * CM], mm_dt)
                        nc.sync.dma_start(
                            wt[:],
                            _ap(wT[t * half:(t + 1) * half],
                                [(CM, 2 * K), (2 * K * CM, half), (1, CM)]))
                        for rp in range(half):
                            first = (t == 0 and rp == 0)
                            last = (t == n_tiles - 1 and rp == half - 1)
                            for h in range(2):
                                nc.tensor.matmul(
                                    s1_psum[:, h * 512:(h + 1) * 512],
                                    xt[:, rp * B:(rp + 1) * B],
                                    wt[:, rp * CM + h * 512:
                                       rp * CM + (h + 1) * 512],
                                    start=first, stop=last,
                                )
                    nc.vector.tensor_copy(s_acc[:], s1_psum[:])
                return s_acc

            def emit_phase1_packed():
                """u production with route-pairs packed on 128 partitions;
                iteration-1 s accumulated on the PE in a dedicated PSUM pair
                via K=128 packed matmuls (u_r0 + u_r1 per pair)."""
                s_acc = sm.tile([B, CM], f32, tag="s_acc")
                n_tiles = R_LOC // RT1
                half = RT1 // 2
                with (
                    tc.tile_pool(name="xp", bufs=3) as xp,
                    tc.tile_pool(name="wp", bufs=3) as wp,
                    tc.tile_pool(name="up1", bufs=3) as up1,
                    tc.tile_pool(name="pp", bufs=3, space="PSUM") as pp,
                    tc.tile_pool(name="s1p", bufs=1, space="PSUM") as s1p,
                ):
                    s1_psum = s1p.tile([B, CM], f32)
                    for t in range(n_tiles):
                        xt = xp.tile([2 * K, half * B], mm_dt)
                        # partition p = k + 64*(r%2); host layout
                        # [rp, par, k, ...] makes (par, k) one stride run
                        nc.sync.dma_start(
                            xt[:],
                            _ap(xT[t * half:(t + 1) * half],
                                [(B, 2 * K), (2 * K * B, half), (1, B)]))
                        wt = wp.tile([2 * K, half * CM], mm_dt)
                        nc.sync.dma_start(
                            wt[:],
                            _ap(wT[t * half:(t + 1) * half],
                                [(CM, 2 * K), (2 * K * CM, half), (1, CM)]))
                        ut = up1.tile([B, RT1 * CM], u_dt)
                        for rp in range(half):
                            first = (t == 0 and rp == 0)
                            last = (t == n_tiles - 1 and rp == half - 1)
                            for h in range(2):
                                # packed: u_even + u_odd accumulated into s1
                                nc.tensor.matmul(
                                    s1_psum[:, h * 512:(h + 1) * 512],
                                    xt[:, rp * B:(rp + 1) * B],
                                    wt[:, rp * CM + h * 512:
                                       rp * CM + (h + 1) * 512],
                                    start=first, stop=last,
                                )
                            for par in range(2):
                                j = 2 * rp + par
                                ps = pp.tile([B, CM], f32)
                                for h in range(2):
                                    nc.tensor.matmul(
                                        ps[:, h * 512:(h + 1) * 512],
                                        xt[par * K:(par + 1) * K,
                                           rp * B:(rp + 1) * B],
                                        wt[par * K:(par + 1) * K,
                                           rp * CM + h * 512:
                                           rp * CM + (h + 1) * 512],
                                        start=True, stop=True,
                                    )
                                if P1COPY == "scalar" or j % 2 == 0:
                                    nc.scalar.copy(
                                        ut[:, j * CM:(j + 1) * CM], ps[:])
                                else:
                                    nc.vector.tensor_copy(
                                        ut[:, j * CM:(j + 1) * CM], ps[:])
                        nc.sync.dma_start(
                            u_dram[:, t * RT1 * CM:(t + 1) * RT1 * CM], ut[:])
                    nc.vector.tensor_copy(s_acc[:], s1_psum[:])
                return s_acc

            def emit_phase1_plain():
                s_acc = sm.tile([B, CM], f32, tag="s_acc")
                nc.vector.memset(s_acc[:], 0.0)
                with (
                    tc.tile_pool(name="xp", bufs=3) as xp,
                    tc.tile_pool(name="wp", bufs=3) as wp,
                    tc.tile_pool(name="up1", bufs=3) as up1,
                    tc.tile_pool(name="pp", bufs=4, space="PSUM") as pp,
                ):
                    for t in range(R_LOC // RT1):
                        xt = xp.tile([K, RT1 * B], mm_dt)
                        nc.sync.dma_start(
                            xt[:],
                            _ap(xT[t * RT1 // 2:(t + 1) * RT1 // 2],
                                [(B, K), (K * B, RT1), (1, B)]))
                        wt = wp.tile([K, RT1 * CM], mm_dt)
                        nc.sync.dma_start(
                            wt[:],
                            _ap(wT[t * RT1 // 2:(t + 1) * RT1 // 2],
                                [(CM, K), (K * CM, RT1), (1, CM)]),
                        )
                        ut = up1.tile([B, RT1 * CM], u_dt)
                        for j in range(RT1):
                            ps = pp.tile([B, CM], f32)
                            for h in range(2):
                                nc.tensor.matmul(
                                    ps[:, h * 512:(h + 1) * 512],
                                    xt[:, j * B:(j + 1) * B],
                                    wt[:, j * CM + h * 512:
                                       j * CM + (h + 1) * 512],
                                    start=True, stop=True,
                                )
                            nc.scalar.copy(ut[:, j * CM:(j + 1) * CM], ps[:])
                        # s1 partial: sum over the tile's routes
                        red = sm.tile([B, CM], f32, tag="red")
                        nc.vector.tensor_reduce(
                            red[:], _view(ut[:], [(1, CM), (CM, RT1)]),
                            axis=AX.X, op=op.add)
                        nc.vector.tensor_tensor(s_acc[:], s_acc[:], red[:],
                                                op=op.add)
                        nc.sync.dma_start(
                            u_dram[:, t * RT1 * CM:(t + 1) * RT1 * CM], ut[:])
                return s_acc

            def emit_once():
                # ------------- Phase 1: u production + iteration-1 s ---------
                if RECOMP:
                    s_acc = emit_s1_only()
                elif S1_ON_PE:
                    s_acc = emit_phase1_packed()
                else:
                    s_acc = emit_phase1_plain()

                allreduce_squash(s_acc, 1.0 / C)

                # ------------- Phase 2: remaining routing iterations ---------
                half2 = RT2 // 2
                with (
                    tc.tile_pool(name="up2",
                                 bufs=int(os.environ.get("DC_UPB", "4"))) as up2,
                    tc.tile_pool(name="pq",
                                 bufs=int(os.environ.get("DC_PQB",
                                                         "1" if RECOMP else "2"))
                                 ) as pq,
                    tc.tile_pool(name="vxp", bufs=1) as vxp,
                    tc.tile_pool(name="cxp", bufs=2) as cxp,
                    tc.tile_pool(name="trp",
                                 bufs=1 if RECOMP else 2) as trp,
                    tc.tile_pool(name="xp2", bufs=3) as xp2,
                    tc.tile_pool(name="wp2", bufs=3) as wp2,
                    tc.tile_pool(name="pp2", bufs=2 if PEMM else 4,
                                 space="PSUM") as pp2,
                    tc.tile_pool(name="dpp", bufs=2, space="PSUM") as dpp,
                    tc.tile_pool(name="spp", bufs=1, space="PSUM") as spp,
                ):
                    n_t2 = R_LOC // RT2
                    def emit_rec(t):
                        """u tile t: PE matmuls from x,w -> PSUM, ACT copies
                        to SBUF fp16. Called one tile AHEAD of consumption so
                        the PE/ACT work pipelines under the DVE routing ops."""
                        ut = up2.tile([B, RT2 * CM], u_dt)
                        xt = xp2.tile([2 * K, half2 * B], mm_dt)
                        nc.sync.dma_start(
                            xt[:],
                            _ap(xT[t * half2:(t + 1) * half2],
                                [(B, 2 * K), (2 * K * B, half2), (1, B)]))
                        wt = wp2.tile([2 * K, half2 * CM], mm_dt)
                        nc.sync.dma_start(
                            wt[:],
                            _ap(wT[t * half2:(t + 1) * half2],
                                [(CM, 2 * K), (2 * K * CM, half2), (1, CM)]))
                        for rp in range(half2):
                            for par in range(2):
                                j = 2 * rp + par
                                ps = pp2.tile([B, CM], f32)
                                for h in range(2):
                                    nc.tensor.matmul(
                                        ps[:, h * 512:(h + 1) * 512],
                                        xt[par * K:(par + 1) * K,
                                           rp * B:(rp + 1) * B],
                                        wt[par * K:(par + 1) * K,
                                           rp * CM + h * 512:
                                           rp * CM + (h + 1) * 512],
                                        start=True, stop=True,
                                    )
                                nc.scalar.copy(
                                    ut[:, j * CM:(j + 1) * CM], ps[:])
                        if u_dram is not None:
                            # hybrid: cache u so later passes DMA-load it
                            # instead of re-running PE matmuls + ACT copies
                            nc.sync.dma_start(
                                u_dram[:, t * RT2 * CM:(t + 1) * RT2 * CM],
                                ut[:])
                        return ut

                    def emit_load(t):
                        ut = up2.tile([B, RT2 * CM], u_dt)
                        nc.sync.dma_start(
                            ut[:], u_dram[:, t * RT2 * CM:(t + 1) * RT2 * CM])
                        return ut

                    if RECOMP and PEMM:
                        # Software-pipelined emission. Per tile iteration:
                        #   stage B(t): p product (DVE) + dot matmuls (PE)
                        #   stage C(t-1): logits/softmax/q/s for the PREVIOUS
                        #     tile, so the DVE queue holds p(t) ahead of the
                        #     ops that wait on PE dot results.
                        # u tiles are emitted two tiles ahead (ut_q).
                        sched = [(it2, t2) for it2 in range(2, n_iters + 1)
                                 for t2 in range(n_t2)]
                        rec_i = [0]
                        ut_q = []

                        def emit_next_rec():
                            if rec_i[0] < len(sched):
                                it2, t2 = sched[rec_i[0]]
                                ut_q.append(emit_rec(t2) if it2 == 2
                                            else emit_load(t2))
                                rec_i[0] += 1

                        def stage_c(it, t, dps, ut, s_psum, last):
                            blt = b_log[:, t * RT2 * C:(t + 1) * RT2 * C]
                            if it == 2:
                                nc.vector.tensor_copy(blt, dps[:])
                            else:
                                nc.vector.tensor_tensor(blt, blt, dps[:],
                                                        op=op.add)
                            mx = sm.tile([B, RT2], f32, tag="mx")
                            nc.vector.tensor_reduce(
                                mx[:], _view(blt, [(C, RT2), (1, C)]),
                                axis=AX.X, op=op.max)
                            e = sm.tile([B, RT2 * C], f32, tag="e")
                            nc.vector.tensor_tensor(
                                _view(e[:], [(C, RT2), (1, C)]),
                                _view(blt, [(C, RT2), (1, C)]),
                                _view(mx[:], [(1, RT2), (0, C)]),
                                op=op.subtract)
                            nc.scalar.activation(
                                e[:], e[:], mybir.ActivationFunctionType.Exp)
                            z = sm.tile([B, RT2], f32, tag="z")
                            nc.vector.tensor_reduce(
                                z[:], _view(e[:], [(C, RT2), (1, C)]),
                                axis=AX.X, op=op.add)
                            nc.vector.reciprocal(z[:], z[:])
                            coef = sm.tile([B, RT2 * C], u_dt, tag="coef")
                            nc.vector.tensor_tensor(
                                _view(coef[:], [(C, RT2), (1, C)]),
                                _view(e[:], [(C, RT2), (1, C)]),
                                _view(z[:], [(1, RT2), (0, C)]),
                                op=op.mult)
                            q = pq.tile([B, RT2 * CM], u_dt, tag="q")
                            nc.vector.tensor_tensor(
                                _view(q[:], [(CM, RT2), (C, M), (1, C)]),
                                _view(ut[:], [(CM, RT2), (C, M), (1, C)]),
                                _view(coef[:], [(C, RT2), (0, M), (1, C)]),
                                op=op.mult)
                            for r2 in range(RT2):
                                for h in range(2):
                                    nc.tensor.matmul(
                                        s_psum[:, h * 512:(h + 1) * 512],
                                        eye[:],
                                        q[:, r2 * CM + h * 512:
                                          r2 * CM + (h + 1) * 512],
                                        start=(t == 0 and r2 == 0),
                                        stop=(last and r2 == RT2 - 1),
                                    )
                            emit_next_rec()

                        emit_next_rec()
                        emit_next_rec()
                        emit_next_rec()
                        for it in range(2, n_iters + 1):
                            s_psum = spp.tile([B, CM], f32, tag="s_psum")
                            carry = None
                            for t in range(n_t2):
                                ut = ut_q.pop(0)
                                p = pq.tile([B, RT2 * CM], u_dt, tag="p")
                                # v broadcast over the tile's routes; innermost
                                # dim contiguous fp16 -> still DVE 2x
                                nc.vector.tensor_tensor(
                                    _view(p[:], [(CM, RT2), (1, CM)]),
                                    _view(ut[:], [(CM, RT2), (1, CM)]),
                                    _view(v_u[:], [(0, RT2), (1, CM)]),
                                    op=op.mult)
                                dps = dpp.tile([B, RT2 * C], f32, tag="dps")
                                for m in range(M):
                                    nc.tensor.matmul(
                                        dps[:], eye[:],
                                        _view(p[:, m * C:],
                                              [(CM, RT2), (1, C)]),
                                        start=(m == 0), stop=(m == M - 1),
                                    )
                                if carry is not None:
                                    stage_c(it, carry[0], carry[1], carry[2],
                                            s_psum, last=False)
                                carry = (t, dps, ut)
                            stage_c(it, carry[0], carry[1], carry[2],
                                    s_psum, last=True)
                            s_acc = sm.tile([B, CM], f32, tag="s_acc")
                            nc.scalar.copy(s_acc[:], s_psum[:])
                            allreduce_squash(s_acc, 1.0)

                    ut_next = emit_rec(0) if RECOMP and not PEMM else None
                    for it in ([] if (RECOMP and PEMM)
                               else range(2, n_iters + 1)):
                        if PEMM:
                            s_psum = spp.tile([B, CM], f32, tag="s_psum")
                        else:
                            s_acc = sm.tile([B, CM], f32, tag="s_acc")
                        if V_EXP:
                            # v expanded over tile routes (ACT, once/pass)
                            v_exp = vxp.tile([B, RT2 * CM], u_dt, tag="v_exp")
                            nc.scalar.copy(
                                v_exp[:], _view(v_sb[:], [(0, RT2), (1, CM)]))
                        for t in range(R_LOC // RT2):
                            if RECOMP:
                                ut = ut_next
                            else:
                                ut = up2.tile([B, RT2 * CM], u_dt)
                                nc.sync.dma_start(
                                    ut[:],
                                    u_dram[:, t * RT2 * CM:(t + 1) * RT2 * CM])
                            p = pq.tile([B, RT2 * CM], u_dt, tag="p")
                            if V_EXP:
                                # both contiguous -> DVE 2x mode
                                prod_p.tensor_tensor(p[:], ut[:], v_exp[:],
                                                     op=op.mult)
                            else:
                                prod_p.tensor_tensor(
                                    _view(p[:], [(CM, RT2), (C, M), (1, C)]),
                                    _view(ut[:], [(CM, RT2), (C, M), (1, C)]),
                                    _view(v_u[:], [(0, RT2), (C, M), (1, C)]),
                                    op=op.mult)
                            # dot[b, (r_t, c)] = sum_m p
                            blt = b_log[:, t * RT2 * C:(t + 1) * RT2 * C]
                            if PEMM:
                                # PE: 32 chained identity-matmuls accumulate
                                # the m-slabs of p into PSUM (exact fp32 sum)
                                dps = dpp.tile([B, RT2 * C], f32, tag="dps")
                                for m in range(M):
                                    nc.tensor.matmul(
                                        dps[:], eye[:],
                                        _view(p[:, m * C:],
                                              [(CM, RT2), (1, C)]),
                                        start=(m == 0), stop=(m == M - 1),
                                    )
                                if it == 2:
                                    nc.scalar.copy(blt, dps[:])
                                else:
                                    nc.vector.tensor_tensor(blt, blt, dps[:],
                                                            op=op.add)
                            elif TREE:
                                _tree_dot(nc, trp, p, blt, it)
                            elif it == 2:
                                nc.vector.tensor_reduce(
                                    blt,
                                    _view(p[:], [(CM, RT2), (1, C), (C, M)]),
                                    axis=AX.X, op=op.add)
                            else:
                                dot = sm.tile([B, RT2 * C], f32, tag="dot")
                                nc.vector.tensor_reduce(
                                    dot[:],
                                    _view(p[:], [(CM, RT2), (1, C), (C, M)]),
                                    axis=AX.X, op=op.add)
                                nc.vector.tensor_tensor(blt, blt, dot[:],
                                                        op=op.add)
                            # softmax over caps (innermost c of blt)
                            mx = sm.tile([B, RT2], f32, tag="mx")
                            nc.vector.tensor_reduce(
                                mx[:], _view(blt, [(C, RT2), (1, C)]),
                                axis=AX.X, op=op.max)
                            e = sm.tile([B, RT2 * C], f32, tag="e")
                            nc.vector.tensor_tensor(
                                _view(e[:], [(C, RT2), (1, C)]),
                                _view(blt, [(C, RT2), (1, C)]),
                                _view(mx[:], [(1, RT2), (0, C)]),
                                op=op.subtract)
                            nc.scalar.activation(
                                e[:], e[:], mybir.ActivationFunctionType.Exp)
                            if RECOMP:
                                # pipeline: next u tile (this pass or tile 0
                                # of the next pass) on PE/ACT while the DVE
                                # finishes softmax + q
                                if t + 1 < n_t2:
                                    ut_next = emit_rec(t + 1)
                                elif it < n_iters:
                                    ut_next = emit_rec(0)
                            z = sm.tile([B, RT2], f32, tag="z")
                            nc.vector.tensor_reduce(
                                z[:], _view(e[:], [(C, RT2), (1, C)]),
                                axis=AX.X, op=op.add)
                            nc.vector.reciprocal(z[:], z[:])
                            coef = sm.tile([B, RT2 * C], u_dt, tag="coef")
                            nc.vector.tensor_tensor(
                                _view(coef[:], [(C, RT2), (1, C)]),
                                _view(e[:], [(C, RT2), (1, C)]),
                                _view(z[:], [(1, RT2), (0, C)]),
                                op=op.mult)
                            q = pq.tile([B, RT2 * CM], u_dt, tag="q")
                            if C_EXP:
                                coef_exp = cxp.tile([B, RT2 * CM], u_dt,
                                                    tag="coef_exp")
                                nc.scalar.copy(
                                    coef_exp[:],
                                    _view(coef[:],
                                          [(C, RT2), (0, M), (1, C)]))
                                prod_q.tensor_tensor(q[:], ut[:],
                                                     coef_exp[:], op=op.mult)
                            elif Q_SWAP:
                                prod_q.tensor_tensor(
                                    _view(q[:], [(CM, RT2), (C, M), (1, C)]),
                                    _view(coef[:],
                                          [(C, RT2), (0, M), (1, C)]),
                                    _view(ut[:], [(CM, RT2), (C, M), (1, C)]),
                                    op=op.mult)
                            else:
                                prod_q.tensor_tensor(
                                    _view(q[:], [(CM, RT2), (C, M), (1, C)]),
                                    _view(ut[:], [(CM, RT2), (C, M), (1, C)]),
                                    _view(coef[:],
                                          [(C, RT2), (0, M), (1, C)]),
                                    op=op.mult)
                            # s partial += sum over r_t of q
                            if PEMM:
                                # PE: accumulate the r-slabs of q into the
                                # pass-level s PSUM (chain spans all tiles)
                                for r2 in range(RT2):
                                    for h in range(2):
                                        nc.tensor.matmul(
                                            s_psum[:, h * 512:(h + 1) * 512],
                                            eye[:],
                                            q[:, r2 * CM + h * 512:
                                              r2 * CM + (h + 1) * 512],
                                            start=(t == 0 and r2 == 0),
                                            stop=(t == n_t2 - 1
                                                  and r2 == RT2 - 1),
                                        )
                            elif TREE:
                                _tree_s(nc, trp, q, s_acc, first=(t == 0))
                            elif t == 0:
                                nc.vector.tensor_reduce(
                                    s_acc[:],
                                    _view(q[:], [(C, M), (1, C), (CM, RT2)]),
                                    axis=AX.X, op=op.add)
                            else:
                                red = sm.tile([B, CM], f32, tag="red")
                                nc.vector.tensor_reduce(
                                    red[:],
                                    _view(q[:], [(C, M), (1, C), (CM, RT2)]),
                                    axis=AX.X, op=op.add)
                                nc.vector.tensor_tensor(s_acc[:], s_acc[:],
                                                        red[:], op=op.add)
                        if PEMM:
                            s_acc = sm.tile([B, CM], f32, tag="s_acc")
                            nc.scalar.copy(s_acc[:], s_psum[:])
                        allreduce_squash(s_acc, 1.0)

            for _ in range(repeat):
                emit_once()

            nc.sync.dma_start(out[:], v_sb[:])

    nc.compile()
    return nc


def _mm_np_dtype():
    if MM_DT == "bfloat16":
        import ml_dtypes

        return ml_dtypes.bfloat16
    if MM_DT == "float16":
        return np.float16
    return np.float32


def make_in_maps(x, w):
    dt = _mm_np_dtype()
    in_maps = []
    for c in range(N_CORES):
        sl = slice(c * R_LOC, (c + 1) * R_LOC)
        xT_c = np.ascontiguousarray(
            x[:, sl, :].transpose(1, 2, 0).reshape(R_LOC // 2, 2, K, B)
        ).astype(dt)
        wT_c = np.ascontiguousarray(
            w[sl].reshape(R_LOC // 2, 2, C, K, M).transpose(0, 1, 3, 4, 2)
        ).reshape(R_LOC // 2, 2, K, CM).astype(dt)
        in_maps.append({"xT": xT_c, "wT": wT_c})
    return in_maps


def kernel(x, route_weights, num_iterations):
    global LAST_RESULT
    from concourse import bass_utils

    n = int(num_iterations)
    assert n >= 1
    x = np.asarray(x, dtype=np.float32)
    w = np.asarray(route_weights, dtype=np.float32)
    assert x.shape == (B, R, K) and w.shape == (R, C, K, M)

    if n not in _compiled:
        _compiled[n] = _build(n)
    nc = _compiled[n]

    in_maps = make_in_maps(x, w)
    res = bass_utils.run_bass_kernel_spmd(
        nc, in_maps, core_ids=list(range(N_CORES)))
    LAST_RESULT = res
    return np.ascontiguousarray(
        res.results[0]["out"].reshape(B, M, C).transpose(0, 2, 1)
    ).astype(np.float32)

